# revision 1
# baseline (speedup 1.0000x reference)
"""Multi-head causal self-attention on 8 TRN2 NeuronCores.

Problem: B=2, T=4096, D=512, H=8 heads (hd=64), fp32 in/out.

Sharding: core c in 0..7 handles batch b = c//4 and head pair g = c%4
(heads 2g, 2g+1 -> D-slice [128g, 128g+128)). Each core computes
    partial_out = concat_h( softmax(causal(Q_h K_h^T / 8)) V_h ) @ W_O[slice]
for its two heads; the host sums the 4 partials per batch and adds b_O.

On-core dataflow (all matmul operands bf16, f32 PSUM accumulation):
  - X^T (host-pretransposed) streams in as 4 [128,4096] chunks.
  - Q^T,K^T [128(d-pair),4096] = W_chunk^T @ X^T, bias added during the
    PSUM->SBUF bf16 copy (per-partition scalar add on DVE).
  - V [4096,64+1] per head in natural layout (ones column appended ->
    the attention row-sum L falls out of the PV matmul for free).
  - Scores are computed transposed, S^T[k-block, q] (contraction over the
    64-dim head axis; the two heads run in disjoint PE row groups), causally
    streamed: for key block kb only q >= 128*kb is computed. exp() runs on
    ScalarE straight out of PSUM with the 1/8 scale folded in; the diagonal
    128x128 subtile is masked by accumulating -1e9 upper-triangle via an
    identity matmul before the exp.
  - Z^T_aug[65, q] accumulates P^T-block x V_aug over key blocks in PSUM;
    row 64 is L. The S->exp->PV chain is software-pipelined: scores run one
    group ahead of the PV matmuls, and PSUM group tiles are triple-buffered,
    which keeps TensorE dense enough that the HAM clock-gate mostly stays at
    2.4 GHz.
  - Normalisation (off the critical path): Z_aug is evacuated to SBUF at
    once (freeing the PSUM bank for the next slice), 1/L runs on a [128,4]
    partition-spread layout (DVE iterative divide costs freedim x 8 cycles),
    and the broadcast across partitions goes through a DRAM bounce (step-0
    partition APs are only legal on DRAM sources).
  - O-projection: lhsT = stacked [Z_A; Z_B] [128, t-tile] (head B shifted to
    partitions 64..127 via SBUF->SBUF DMA), rhs = W_O pair [128,512].
    Each slice's O-projection is emitted AFTER the next slice's attention
    groups so its normalisation chain never stalls the PE stream.
"""

import numpy as np

import concourse.bass as bass
import concourse.mybir as mybir
from concourse.tile import TileContext
from concourse.bass_utils import run_bass_kernel_spmd

try:
    import ml_dtypes

    _BF16 = ml_dtypes.bfloat16
except ImportError:  # pragma: no cover
    _BF16 = None

F32 = mybir.dt.float32
BF16 = mybir.dt.bfloat16

B, T, D, H = 2, 4096, 512, 8
HD = D // H  # 64
SW = 512  # q-slice width
NS = T // SW  # 8 q-slices
NKC = D // 128  # 4 contraction chunks for the projections
NTT = T // 128  # 32 t-tiles / key blocks
GK = 2  # key blocks grouped per exp() call (2 PSUM banks)
NEG = -1.0e9


def _split_waits(nc, max_waits=1):
    """The staged walrus rejects >1 semaphore wait per instruction; hoist
    extras onto same-engine NoOps inserted right before the instruction."""
    counter = 0
    for f in nc.m.functions:
        for blk in f.blocks:
            insts = blk.instructions
            out, changed = [], False
            for ins in insts:
                si = getattr(ins, "sync_info", None)
                waits = list(si.on_wait) if si is not None and si.on_wait else []
                if len(waits) > max_waits:
                    changed = True
                    for w in waits[:-max_waits]:
                        counter += 1
                        nop = mybir.InstNoOp(
                            name=f"I-wsplit-{counter}",
                            engine=ins.engine,
                            ins=[],
                            outs=[],
                        )
                        nop.sync_info = mybir.SyncInfo(on_wait=[w], on_update=[])
                        out.append(nop)
                    ins.sync_info = mybir.SyncInfo(
                        on_wait=waits[-max_waits:], on_update=list(si.on_update)
                    )
                out.append(ins)
            if changed:
                blk.instructions = out
    return counter


def build_nc():
    nc = bass.Bass("TRN2")

    xt = nc.dram_tensor("xt", [D, T], BF16, kind="ExternalInput")
    wq = nc.dram_tensor("wq", [D, 128], BF16, kind="ExternalInput")
    wk = nc.dram_tensor("wk", [D, 128], BF16, kind="ExternalInput")
    wv = nc.dram_tensor("wv", [D, 128], BF16, kind="ExternalInput")
    wo = nc.dram_tensor("wo", [128, D], BF16, kind="ExternalInput")
    bq = nc.dram_tensor("bq", [128, 1], F32, kind="ExternalInput")
    bk = nc.dram_tensor("bk", [128, 1], F32, kind="ExternalInput")
    bv = nc.dram_tensor("bv", [1, 128], BF16, kind="ExternalInput")
    out = nc.dram_tensor("out", [T, D], F32, kind="ExternalOutput")

    ident_np = np.eye(128, dtype=np.float32)
    # maskneg[k, q'] = 0 where q' >= k else NEG  (S^T diagonal subtile mask)
    mask_np = np.where(
        np.arange(128)[None, :] >= np.arange(128)[:, None], 0.0, NEG
    ).astype(np.float32)
    ident_dram = nc.inline_tensor(ident_np.astype(_BF16), name="identc")
    mask_dram = nc.inline_tensor(mask_np.astype(_BF16), name="maskc")

    with TileContext(nc) as tc:
        with (
            tc.tile_pool(name="singles", bufs=1) as singles,
            tc.tile_pool(name="ps", bufs=3, space="PSUM") as ps,
            tc.tile_pool(name="zps", bufs=1, space="PSUM") as zps,
            tc.tile_pool(name="pt", bufs=6) as ptp,
            tc.tile_pool(name="sl", bufs=3) as slp,
            tc.tile_pool(name="outp", bufs=4) as outp,
            tc.tile_pool(name="drp", bufs=2, space="DRAM") as drp,
        ):
            # ---- static SBUF ----
            xt_sb = [
                [
                    singles.tile(
                        [128, SW], BF16, tag=f"xt{c}_{s}", name=f"xt_sb{c}_{s}"
                    )
                    for s in range(NS)
                ]
                for c in range(NKC)
            ]
            for s in range(NS):
                for c in range(NKC):
                    nc.sync.dma_start(
                        out=xt_sb[c][s][:, :],
                        in_=xt[c * 128 : (c + 1) * 128, s * SW : (s + 1) * SW],
                    )

            wq_sb = singles.tile([128, NKC, 128], BF16, tag="wq")
            wk_sb = singles.tile([128, NKC, 128], BF16, tag="wk")
            wv_sb = singles.tile([128, NKC, 128], BF16, tag="wv")
            for c in range(NKC):
                nc.sync.dma_start(out=wq_sb[:, c, :], in_=wq[c * 128 : (c + 1) * 128, :])
                nc.sync.dma_start(out=wk_sb[:, c, :], in_=wk[c * 128 : (c + 1) * 128, :])
                nc.sync.dma_start(out=wv_sb[:, c, :], in_=wv[c * 128 : (c + 1) * 128, :])
            wo_sb = singles.tile([128, D], BF16, tag="wo")
            nc.sync.dma_start(out=wo_sb[:, :], in_=wo[:, :])

            bq_sb = singles.tile([128, 1], F32, tag="bq")
            bk_sb = singles.tile([128, 1], F32, tag="bk")
            bv_sb = singles.tile([1, 512], BF16, tag="bv")
            nc.sync.dma_start(out=bq_sb[:, :], in_=bq[:, :])
            nc.sync.dma_start(out=bk_sb[:, :], in_=bk[:, :])
            for j in range(4):
                nc.sync.dma_start(out=bv_sb[:, j * 128 : (j + 1) * 128], in_=bv[:, :])

            ident_sb = singles.tile([128, 128], BF16, tag="ident")
            mask_sb = singles.tile([128, 128], BF16, tag="mask")
            nc.sync.dma_start(out=ident_sb[:, :], in_=ident_dram[:, :])
            nc.sync.dma_start(out=mask_sb[:, :], in_=mask_dram[:, :])
            ones_sb = singles.tile([1, 128], BF16, tag="ones")
            nc.vector.memset(ones_sb[:, :], 1.0)

            qt_sb = [
                singles.tile([128, SW], BF16, tag=f"qt{s}", name=f"qt_sb{s}")
                for s in range(NS)
            ]
            kt_sb = [
                singles.tile([128, SW], BF16, tag=f"kt{s}", name=f"kt_sb{s}")
                for s in range(NS)
            ]
            # V_aug per head per key block: [128(t), 65]; col 64 = ones
            va_sb = [
                singles.tile([128, HD + 1], BF16, tag=f"va{t}", name=f"va_sb{t}")
                for t in range(NTT)
            ]
            vb_sb = [
                singles.tile([128, HD + 1], BF16, tag=f"vb{t}", name=f"vb_sb{t}")
                for t in range(NTT)
            ]
            ones_row = singles.tile([1, SW], F32, tag="onesrow")
            nc.vector.memset(ones_row[:, :], 1.0)

            # ---- QKV projections (emitted per q-slice, interleaved with
            # attention so ScalarE starts exp-ing early) ----
            def emit_qkv(s):
                cols = slice(s * SW, (s + 1) * SW)
                ps_q = ps.tile([128, SW], F32, tag="sg", name="ps_q")
                for c in range(NKC):
                    nc.tensor.matmul(
                        ps_q[:, :],
                        lhsT=wq_sb[:, c, :],
                        rhs=xt_sb[c][s][:, :],
                        start=(c == 0),
                        stop=(c == NKC - 1),
                        skip_group_check=True,
                    )
                nc.vector.tensor_scalar_add(qt_sb[s][:, :], ps_q[:, :], bq_sb[:, :])
                ps_k = ps.tile([128, SW], F32, tag="sg", name="ps_k")
                for c in range(NKC):
                    nc.tensor.matmul(
                        ps_k[:, :],
                        lhsT=wk_sb[:, c, :],
                        rhs=xt_sb[c][s][:, :],
                        start=(c == 0),
                        stop=(c == NKC - 1),
                        skip_group_check=True,
                    )
                nc.vector.tensor_scalar_add(kt_sb[s][:, :], ps_k[:, :], bk_sb[:, :])
                for t in range(4 * s, 4 * s + 4):
                    tloc = slice((t % 4) * 128, (t % 4 + 1) * 128)
                    ps_v = ps.tile([128, 128], F32, tag="sg", name="ps_v")
                    for c in range(NKC):
                        nc.tensor.matmul(
                            ps_v[:, :],
                            lhsT=xt_sb[c][s][:, tloc],
                            rhs=wv_sb[:, c, :],
                            start=(c == 0),
                            stop=False,
                            skip_group_check=True,
                        )
                    # + b_V broadcast over rows:  ones[1,128]^T @ bv[1,128]
                    nc.tensor.matmul(
                        ps_v[:, :],
                        lhsT=ones_sb[:, :],
                        rhs=bv_sb[:, 0:128],
                        start=False,
                        stop=True,
                        skip_group_check=True,
                    )
                    nc.vector.tensor_copy(va_sb[t][:, 0:HD], ps_v[:, 0:HD])
                    nc.vector.tensor_copy(vb_sb[t][:, 0:HD], ps_v[:, HD:128])
                    nc.vector.memset(va_sb[t][:, HD : HD + 1], 1.0)
                    nc.vector.memset(vb_sb[t][:, HD : HD + 1], 1.0)

            # ---- attention ----
            vmat = (va_sb, vb_sb)
            hrows = (slice(0, HD), slice(HD, 128))

            def emit_oproj(znpair_t, qs_t):
                for j in range(4):
                    ps_o = ps.tile([128, D], F32, tag="sg", name="ps_o")
                    nc.tensor.matmul(
                        ps_o[:, :],
                        lhsT=znpair_t[:, j * 128 : (j + 1) * 128],
                        rhs=wo_sb[:, :],
                        start=True,
                        stop=True,
                        skip_group_check=True,
                    )
                    o_sb = outp.tile([128, D], F32, tag="ot", name="o_sb")
                    nc.vector.tensor_copy(o_sb[:, :], ps_o[:, :])
                    r0 = qs_t + j * 128
                    nc.sync.dma_start(out=out[r0 : r0 + 128, :], in_=o_sb[:, :])

            pending = []
            for s in range(NS):
                emit_qkv(s)
                qs = s * SW
                nkb = 4 * (s + 1)
                zaug = [
                    zps.tile([HD + 1, SW], F32, tag="za", name="zauga"),
                    zps.tile([HD + 1, SW], F32, tag="zb", name="zaugb"),
                ]
                # pack key blocks tightly into groups; a matmul output may
                # not cross a PSUM bank boundary, so bump to the next bank
                # when a block would straddle one
                groups, cur, cur_cols = [], [], 0
                for kb in range(nkb):
                    qlo = max(qs, kb * 128)
                    n = qs + SW - qlo
                    off = cur_cols
                    if off % SW + n > SW:
                        off = ((off + SW - 1) // SW) * SW
                    if off + n > GK * SW:
                        groups.append(cur)
                        cur, off = [], 0
                    cur.append((kb, off, n, qlo))
                    cur_cols = off + n
                if cur:
                    groups.append(cur)
                def emit_av(av):
                    pt_t, grp_t = av
                    for h in range(2):
                        for kb, off, n, qlo in grp_t:
                            nc.tensor.matmul(
                                zaug[h][0 : HD + 1, qlo - qs : SW],
                                lhsT=vmat[h][kb][:, :],
                                rhs=pt_t[h][:, off : off + n],
                                start=(kb == 0),
                                stop=(kb == nkb - 1),
                                skip_group_check=True,
                            )

                av_queue = []
                for grp in groups:
                    used = grp[-1][1] + grp[-1][2]
                    sg = [None, None]
                    pt = [None, None]
                    for h in range(2):
                        sg[h] = ps.tile([128, GK * SW], F32, tag="sg", name="sg")
                        pt[h] = ptp.tile([128, GK * SW], BF16, tag="pt", name="pt")
                    # scores (both heads interleaved -> disjoint PE row groups)
                    for kb, off, n, qlo in grp:
                        diag = kb * 128 >= qs
                        for h in range(2):
                            nc.tensor.matmul(
                                sg[h][:, off : off + n],
                                lhsT=kt_sb[kb // 4][hrows[h], (kb % 4) * 128 : (kb % 4 + 1) * 128],
                                rhs=qt_sb[s][hrows[h], qlo - qs : qlo - qs + n],
                                start=True,
                                stop=not diag,
                                skip_group_check=True,
                                tile_position=(h * HD, 0),
                            )
                        if diag:
                            for h in range(2):
                                nc.tensor.matmul(
                                    sg[h][:, off : off + 128],
                                    lhsT=ident_sb[:, :],
                                    rhs=mask_sb[:, :],
                                    start=False,
                                    stop=True,
                                    skip_group_check=True,
                                )
                    for h in range(2):
                        nc.scalar.activation(
                            out=pt[h][:, 0:used],
                            in_=sg[h][:, 0:used],
                            func=mybir.ActivationFunctionType.Exp,
                            scale=0.125,
                        )
                    av_queue.append((pt, grp))
                    if len(av_queue) > 1:
                        emit_av(av_queue.pop(0))
                while av_queue:
                    emit_av(av_queue.pop(0))

                # earlier slices' O-projections: their normalisation chains
                # have had >=1 slice of compute to finish -> PE never stalls
                if s >= 2:
                    for p in pending:
                        emit_oproj(*p)
                    pending = []

                last = s == NS - 1
                # evacuate Z^T_aug to SBUF right away (frees the PSUM bank);
                # L row lands at partition 0 so GpSimd ops are partition-aligned.
                # For the final slice latency matters more than PSUM reuse, so
                # work straight off PSUM and use the one-shot reciprocal.
                zsb = [None, None]
                lrow = [None, None]
                if not last:
                    for h in range(2):
                        zsb[h] = slp.tile([HD, SW], F32, tag=f"zsb{h}", name="zsb")
                        nc.vector.tensor_copy(zsb[h][:, :], zaug[h][0:HD, :])
                        lrow[h] = slp.tile([1, SW], F32, tag=f"lr{h}", name="lrow")
                        nc.vector.tensor_copy(lrow[h][:, :], zaug[h][HD : HD + 1, :])

                # normalise z[:, q] / L[q]; the reciprocal runs on a
                # [128, 4] partition-spread layout (DVE iterative divide
                # costs free-dim x 8 cycles, so spread the 512 elements)
                znpair = slp.tile([128, SW], BF16, tag="zn")
                znb = slp.tile([HD, SW], BF16, tag="znb")
                for h in range(2):
                    rd2 = drp.tile([1, SW], F32, tag=f"rd2{h}", name="rd2")
                    if last:
                        rsp1 = slp.tile([1, SW], F32, tag=f"rs1{h}", name="rsp1")
                        nc.vector.reciprocal(rsp1[:, :], zaug[h][HD : HD + 1, :])
                        nc.sync.dma_start(out=rd2[:, :], in_=rsp1[:, :])
                    else:
                        rd = drp.tile([1, SW], F32, tag=f"rd{h}", name="rd")
                        nc.sync.dma_start(out=rd[:, :], in_=lrow[h][:, :])
                        lsp = slp.tile(
                            [128, SW // 128], F32, tag=f"lsp{h}", name="lsp"
                        )
                        nc.sync.dma_start(
                            out=lsp[:, :],
                            in_=rd[0, :].rearrange("(p f) -> p f", p=128),
                        )
                        rsp = slp.tile(
                            [128, SW // 128], F32, tag=f"rsp{h}", name="rsp"
                        )
                        nc.vector.reciprocal(rsp[:, :], lsp[:, :])
                        nc.sync.dma_start(
                            out=rd2[0, :].rearrange("(p f) -> p f", p=128),
                            in_=rsp[:, :],
                        )
                    bc = slp.tile([HD, SW], F32, tag=f"bc{h}")
                    rap = rd2[:, :]
                    bcast_src = bass.AP(
                        tensor=rap.tensor,
                        offset=rap.offset,
                        ap=[[0, HD]] + list(rap.ap[1:]),
                    )
                    nc.sync.dma_start(out=bc[:, :], in_=bcast_src)
                    dst = znpair[0:HD, :] if h == 0 else znb[:, :]
                    src_z = zaug[h][0:HD, :] if last else zsb[h][:, :]
                    nc.vector.tensor_mul(dst, src_z, bc[:, :])
                # move head B rows into partitions 64..127
                nc.gpsimd.dma_start(out=znpair[HD:128, :], in_=znb[:, :])
                pending.append((znpair, qs))

            for p in pending:
                emit_oproj(*p)

    _split_waits(nc)
    return nc


_NC_CACHE = {}


def _get_nc():
    if "nc" not in _NC_CACHE:
        _NC_CACHE["nc"] = build_nc()
    return _NC_CACHE["nc"]


def make_in_maps(combined_embed, W_K, b_K, W_Q, b_Q, W_V, b_V, W_O, b_O):
    f32 = np.float32
    in_maps = []
    for c in range(8):
        b = c // 4
        g = c % 4
        sl = slice(g * 128, (g + 1) * 128)
        xt = np.ascontiguousarray(np.asarray(combined_embed[b], f32).T)
        in_maps.append(
            {
                "xt": xt.astype(_BF16),
                "wq": np.ascontiguousarray(np.asarray(W_Q, f32)[:, sl]).astype(_BF16),
                "wk": np.ascontiguousarray(np.asarray(W_K, f32)[:, sl]).astype(_BF16),
                "wv": np.ascontiguousarray(np.asarray(W_V, f32)[:, sl]).astype(_BF16),
                "wo": np.ascontiguousarray(np.asarray(W_O, f32)[sl, :]).astype(_BF16),
                "bq": np.asarray(b_Q, f32)[sl].reshape(128, 1).copy(),
                "bk": np.asarray(b_K, f32)[sl].reshape(128, 1).copy(),
                "bv": np.asarray(b_V, f32)[sl].reshape(1, 128).astype(_BF16),
            }
        )
    return in_maps


def run_cores(in_maps, **kwargs):
    nc = _get_nc()
    return run_bass_kernel_spmd(nc, in_maps, core_ids=list(range(8)), **kwargs)


def kernel(
    combined_embed, W_K, b_K, W_Q, b_Q, W_V, b_V, W_O, b_O
):  # full inputs -> full output
    in_maps = make_in_maps(
        combined_embed, W_K, b_K, W_Q, b_Q, W_V, b_V, W_O, b_O
    )
    res = run_cores(in_maps)
    out = np.zeros((B, T, D), np.float32)
    for c in range(8):
        out[c // 4] += res.results[c]["out"]
    out += np.asarray(b_O, np.float32)[None, None, :]
    return out



# revision 20
# speedup vs baseline: 1.5674x; 1.5674x over previous
"""Multi-head causal self-attention on 8 TRN2 NeuronCores.

Problem: B=2, T=4096, D=512, H=8 heads (hd=64), fp32 in/out.

Sharding: core c in 0..7 handles batch b = c//4 and head pair g = c%4
(heads 2g, 2g+1 -> D-slice [128g, 128g+128)). Each core computes
    partial_out = concat_h( softmax(causal(Q_h K_h^T / 8)) V_h ) @ W_O[slice]
for its two heads; the host sums the 4 partials per batch and adds b_O.

On-core dataflow (all matmul operands bf16, f32 PSUM accumulation):
  - X^T (host-pretransposed) streams in as 4 [128,4096] chunks.
  - Q^T,K^T [128(d-pair),4096] = W_chunk^T @ X^T, bias added during the
    PSUM->SBUF bf16 copy (per-partition scalar add on DVE).
  - V [4096,64+1] per head in natural layout (ones column appended ->
    the attention row-sum L falls out of the PV matmul for free); the
    V bias rides on the PSUM->SBUF evacuation (DVE tensor_tensor add
    against a partition-broadcast bias tile) so the PE never sees it.
  - Scores are computed transposed, S^T[k-block, q] (contraction over the
    64-dim head axis; the two heads run in disjoint PE row groups), causally
    streamed: for key block kb only q >= 128*kb is computed. exp() runs on
    ScalarE straight out of PSUM with the 1/8 scale folded in; the diagonal
    128x128 subtile is masked AFTER the exp by a DVE multiply with a 0/1
    lower-triangle tile (keeps the PE stream free of mask matmuls and of
    tiling-mode switches mid-group).
  - Z^T_aug[65, q] accumulates P^T-block x V_aug over key blocks in PSUM;
    row 64 is L. The S->exp->PV chain is software-pipelined: scores run one
    group ahead of the PV matmuls; score PSUM groups are double-buffered and
    Z^T PSUM is double-buffered across slices so the PE never waits on the
    normalisation chain.
  - Normalisation (fully off the PE): head B's Z rows are DMA-shifted from
    PSUM into partitions 64..127 of an f32 staging tile while the L rows
    bounce through DRAM into a [128,4] partition-spread layout (DVE
    iterative divide costs freedim x 8 cycles, so spread the 512 elements),
    are reciprocal'd, bounced back, and broadcast-read into a [128,512]
    tile (step-0 partition APs are only legal on DRAM sources). Two DVE
    multiplies then produce the normalised bf16 [128,512] O-proj lhsT
    directly (head A straight out of PSUM).
  - O-projection: lhsT = stacked [Z_A; Z_B] [128, t-tile], rhs = W_O pair
    [128,512]. Each slice's O-projection is emitted at the END of the next
    slice's body so its normalisation chain never stalls the PE stream and
    its PSUM->SBUF copies never delay the next slice's DVE work.
"""

import numpy as np

import concourse.bass as bass
import concourse.mybir as mybir
from concourse.tile import TileContext
from concourse.bass_utils import run_bass_kernel_spmd

try:
    import ml_dtypes

    _BF16 = ml_dtypes.bfloat16
except ImportError:  # pragma: no cover
    _BF16 = None

F32 = mybir.dt.float32
BF16 = mybir.dt.bfloat16

B, T, D, H = 2, 4096, 512, 8
HD = D // H  # 64
SW = 512  # q-slice width
NS = T // SW  # 8 q-slices
NKC = D // 128  # 4 contraction chunks for the projections
NTT = T // 128  # 32 t-tiles / key blocks
GK = 2  # key blocks grouped per exp() call (2 PSUM banks)


def _split_waits(nc, max_waits=1):
    """The staged walrus rejects >1 semaphore wait per instruction; hoist
    extras onto same-engine NoOps inserted right before the instruction."""
    counter = 0
    for f in nc.m.functions:
        for blk in f.blocks:
            insts = blk.instructions
            out, changed = [], False
            for ins in insts:
                si = getattr(ins, "sync_info", None)
                waits = list(si.on_wait) if si is not None and si.on_wait else []
                if len(waits) > max_waits:
                    changed = True
                    for w in waits[:-max_waits]:
                        counter += 1
                        nop = mybir.InstNoOp(
                            name=f"I-wsplit-{counter}",
                            engine=ins.engine,
                            ins=[],
                            outs=[],
                        )
                        nop.sync_info = mybir.SyncInfo(on_wait=[w], on_update=[])
                        out.append(nop)
                    ins.sync_info = mybir.SyncInfo(
                        on_wait=waits[-max_waits:], on_update=list(si.on_update)
                    )
                out.append(ins)
            if changed:
                blk.instructions = out
    return counter


def build_nc():
    nc = bass.Bass("TRN2")

    xt = nc.dram_tensor("xt", [D, T], BF16, kind="ExternalInput")
    wq = nc.dram_tensor("wq", [D, 128], BF16, kind="ExternalInput")
    wk = nc.dram_tensor("wk", [D, 128], BF16, kind="ExternalInput")
    wv = nc.dram_tensor("wv", [D, 128], BF16, kind="ExternalInput")
    wo = nc.dram_tensor("wo", [128, D], BF16, kind="ExternalInput")
    bq = nc.dram_tensor("bq", [128, 1], F32, kind="ExternalInput")
    bk = nc.dram_tensor("bk", [128, 1], F32, kind="ExternalInput")
    bv = nc.dram_tensor("bv", [1, 128], F32, kind="ExternalInput")
    out = nc.dram_tensor("out", [T, D], F32, kind="ExternalOutput")

    # tri01[k, q'] = 1 where q' >= k else 0  (post-exp S^T diagonal mask)
    tri_np = np.where(
        np.arange(128)[None, :] >= np.arange(128)[:, None], 1.0, 0.0
    ).astype(np.float32)
    tri_dram = nc.inline_tensor(tri_np.astype(_BF16), name="tric")
    ident_dram = nc.inline_tensor(np.eye(128, dtype=np.float32), name="identc")

    with TileContext(nc) as tc:
        with (
            tc.tile_pool(name="singles", bufs=1) as singles,
            tc.tile_pool(name="ps", bufs=3, space="PSUM") as ps,
            tc.tile_pool(name="zps", bufs=1, space="PSUM") as zps,
            tc.tile_pool(name="pt", bufs=6) as ptp,
            tc.tile_pool(name="sl", bufs=3) as slp,
            tc.tile_pool(name="outp", bufs=4) as outp,
            tc.tile_pool(name="drp", bufs=2, space="DRAM") as drp,
        ):
            # ---- static SBUF ----
            # DMA issue is serialized per queue (~0.7us each), so: the handful
            # the first projections need go on the sync queue first (QKV
            # weights as ONE consolidated DMA each + slice 0 of X^T), the X^T
            # bulk goes on the gpsimd queue, everything else on scalar/vector.
            xt_sb = [
                singles.tile([128, NS, SW], BF16, tag=f"xt{c}", name=f"xt_sb{c}")
                for c in range(NKC)
            ]
            wq_sb = singles.tile([128, NKC, 128], BF16, tag="wq")
            wk_sb = singles.tile([128, NKC, 128], BF16, tag="wk")
            wv_sb = singles.tile([128, NKC, 128], BF16, tag="wv")
            nc.sync.dma_start(
                out=wq_sb[:, :, :],
                in_=wq[:, :].rearrange("(c p) f -> p c f", p=128),
            )
            nc.sync.dma_start(
                out=wk_sb[:, :, :],
                in_=wk[:, :].rearrange("(c p) f -> p c f", p=128),
            )
            nc.sync.dma_start(
                out=wv_sb[:, :, :],
                in_=wv[:, :].rearrange("(c p) f -> p c f", p=128),
            )
            for c in range(NKC):
                nc.sync.dma_start(
                    out=xt_sb[c][:, 0, :],
                    in_=xt[c * 128 : (c + 1) * 128, 0:SW],
                )
            for si in range(1, NS):
                for c in range(NKC):
                    nc.gpsimd.dma_start(
                        out=xt_sb[c][:, si, :],
                        in_=xt[c * 128 : (c + 1) * 128, si * SW : (si + 1) * SW],
                    )

            bq_sb = singles.tile([128, 1], F32, tag="bq")
            bk_sb = singles.tile([128, 1], F32, tag="bk")
            nc.scalar.dma_start(out=bq_sb[:, :], in_=bq[:, :])
            nc.scalar.dma_start(out=bk_sb[:, :], in_=bk[:, :])

            wo_sb = singles.tile([128, D], BF16, tag="wo")
            nc.scalar.dma_start(out=wo_sb[:, :], in_=wo[:, :])
            # W_O rows 64..127 restaged at partitions 0..63 for the tail's
            # per-head 64-contraction O-projections
            wob_sb = singles.tile([HD, D], BF16, tag="wob")
            nc.scalar.dma_start(out=wob_sb[:, :], in_=wo[HD:128, :])
            # b_V broadcast across partitions: [128, 128] f32, every row = b_V
            bvb_sb = singles.tile([128, 128], F32, tag="bvb")
            bv_ap = bv[:, :]
            bvb_src = bass.AP(
                tensor=bv_ap.tensor,
                offset=bv_ap.offset,
                ap=[[0, 128]] + list(bv_ap.ap[1:]),
            )
            nc.scalar.dma_start(out=bvb_sb[:, :], in_=bvb_src)

            tri_sb = singles.tile([128, 128], BF16, tag="tri")
            nc.scalar.dma_start(out=tri_sb[:, :], in_=tri_dram[:, :])
            ident_sb = singles.tile([128, 128], F32, tag="ident")
            nc.scalar.dma_start(out=ident_sb[:, :], in_=ident_dram[:, :])

            qt_sb = [
                singles.tile([128, SW], BF16, tag=f"qt{s}", name=f"qt_sb{s}")
                for s in range(NS)
            ]
            kt_sb = [
                singles.tile([128, SW], BF16, tag=f"kt{s}", name=f"kt_sb{s}")
                for s in range(NS)
            ]
            # V_aug per head per key block: [128(t), 65]; col 64 = ones
            va_sb = [
                singles.tile([128, HD + 1], BF16, tag=f"va{t}", name=f"va_sb{t}")
                for t in range(NTT)
            ]
            vb_sb = [
                singles.tile([128, HD + 1], BF16, tag=f"vb{t}", name=f"vb_sb{t}")
                for t in range(NTT)
            ]

            # ---- QKV projections (emitted per q-slice, interleaved with
            # attention so ScalarE starts exp-ing early) ----
            def emit_qkv(s):
                ps_q = ps.tile([128, SW], F32, tag="sg", name="ps_q")
                for c in range(NKC):
                    nc.tensor.matmul(
                        ps_q[:, :],
                        lhsT=wq_sb[:, c, :],
                        rhs=xt_sb[c][:, s, :],
                        start=(c == 0),
                        stop=(c == NKC - 1),
                        skip_group_check=True,
                    )
                nc.vector.tensor_scalar_add(qt_sb[s][:, :], ps_q[:, :], bq_sb[:, :])
                ps_k = ps.tile([128, SW], F32, tag="sg", name="ps_k")
                for c in range(NKC):
                    nc.tensor.matmul(
                        ps_k[:, :],
                        lhsT=wk_sb[:, c, :],
                        rhs=xt_sb[c][:, s, :],
                        start=(c == 0),
                        stop=(c == NKC - 1),
                        skip_group_check=True,
                    )
                nc.vector.tensor_scalar_add(kt_sb[s][:, :], ps_k[:, :], bk_sb[:, :])
                for t in range(4 * s, 4 * s + 4):
                    tloc = slice((t % 4) * 128, (t % 4 + 1) * 128)
                    ps_v = ps.tile([128, 128], F32, tag="sg", name="ps_v")
                    for c in range(NKC):
                        nc.tensor.matmul(
                            ps_v[:, :],
                            lhsT=xt_sb[c][:, s, tloc],
                            rhs=wv_sb[:, c, :],
                            start=(c == 0),
                            stop=(c == NKC - 1),
                            skip_group_check=True,
                        )
                    nc.vector.tensor_tensor(
                        va_sb[t][:, 0:HD],
                        ps_v[:, 0:HD],
                        bvb_sb[:, 0:HD],
                        op=mybir.AluOpType.add,
                    )
                    nc.vector.tensor_tensor(
                        vb_sb[t][:, 0:HD],
                        ps_v[:, HD:128],
                        bvb_sb[:, HD:128],
                        op=mybir.AluOpType.add,
                    )
                    nc.vector.memset(va_sb[t][:, HD : HD + 1], 1.0)
                    nc.vector.memset(vb_sb[t][:, HD : HD + 1], 1.0)

            # ---- attention ----
            vmat = (va_sb, vb_sb)
            hrows = (slice(0, HD), slice(HD, 128))

            def emit_oproj(znpair_t, qs_t):
                for j in range(4):
                    ps_o = ps.tile([128, D], F32, tag="sg", name="ps_o")
                    nc.tensor.matmul(
                        ps_o[:, :],
                        lhsT=znpair_t[:, j * 128 : (j + 1) * 128],
                        rhs=wo_sb[:, :],
                        start=True,
                        stop=True,
                        skip_group_check=True,
                    )
                    o_sb = outp.tile([128, D], F32, tag="ot", name="o_sb")
                    nc.vector.tensor_copy(o_sb[:, :], ps_o[:, :])
                    r0 = qs_t + j * 128
                    nc.sync.dma_start(out=out[r0 : r0 + 128, :], in_=o_sb[:, :])

            pending = []
            for s in range(NS):
                emit_qkv(s)
                qs = s * SW
                nkb = 4 * (s + 1)
                zaug = [
                    zps.tile([HD + 1, SW], F32, tag="za", name="zauga"),
                    zps.tile([HD + 1, SW], F32, tag="zb", name="zaugb"),
                ]

                def emit_av(av):
                    pt_t, kb_t, n_t, qlo_t = av
                    for h in range(2):
                        nc.tensor.matmul(
                            zaug[h][0 : HD + 1, qlo_t - qs : SW],
                            lhsT=vmat[h][kb_t][:, :],
                            rhs=pt_t[:, h, 0:n_t],
                            start=(kb_t == 0),
                            stop=(kb_t == nkb - 1),
                            skip_group_check=True,
                        )

                # head-major score groups: one [128, 2, 512] PSUM tile per
                # key block (head A in bank 0, head B in bank 1) -> three
                # kb-groups in flight through the S -> exp -> PV pipeline
                av_queue = []
                for kb in range(nkb):
                    qlo = max(qs, kb * 128)
                    n = qs + SW - qlo
                    diag = kb * 128 >= qs
                    sg = ps.tile([128, 2, SW], F32, tag="sg", name="sg")
                    pt = ptp.tile([128, 2, SW], BF16, tag="pt", name="pt")
                    # scores (both heads -> disjoint PE row groups)
                    for h in range(2):
                        nc.tensor.matmul(
                            sg[:, h, 0:n],
                            lhsT=kt_sb[kb // 4][hrows[h], (kb % 4) * 128 : (kb % 4 + 1) * 128],
                            rhs=qt_sb[s][hrows[h], qlo - qs : qlo - qs + n],
                            start=True,
                            stop=True,
                            skip_group_check=True,
                            tile_position=(h * HD, 0),
                        )
                    nc.scalar.activation(
                        out=pt[:, :, 0:n],
                        in_=sg[:, :, 0:n],
                        func=mybir.ActivationFunctionType.Exp,
                        scale=0.125,
                    )
                    if diag:
                        # post-exp causal mask on the diagonal 128x128 subtile
                        for h in range(2):
                            nc.vector.tensor_mul(
                                pt[:, h, 0:128],
                                pt[:, h, 0:128],
                                tri_sb[:, :],
                            )
                    av_queue.append((pt, kb, n, qlo))
                    if len(av_queue) > 2:
                        emit_av(av_queue.pop(0))
                while av_queue:
                    emit_av(av_queue.pop(0))

                # ---- normalisation (no PE work; zaug evacuated by ScalarE
                # right after the last PV so the single-buffered Z PSUM bank
                # frees before the next slice's PV needs it) ----
                zsb = [None, None]
                for h in range(2):
                    zsb[h] = slp.tile([HD + 1, SW], F32, tag=f"zsb{h}", name="zsb")
                    nc.scalar.activation(
                        out=zsb[h][:, :],
                        in_=zaug[h][:, :],
                        func=mybir.ActivationFunctionType.Copy,
                    )

                if s == NS - 1:
                    # ---- low-latency tail: skip the DRAM 1/L bounce.
                    # L rows -> per-partition layout via PE transposes (PE is
                    # idle here), one DVE reciprocal, then per-head 64-contraction
                    # O-projections whose 1/L scale rides on the PSUM->SBUF
                    # evacuation (tensor_scalar ops, per-partition scalars).
                    if pending:  # give the PE work while the chain drains
                        for p in pending:
                            emit_oproj(*p)
                        pending = []
                    zna = slp.tile([HD, SW], BF16, tag="zn", name="zna")
                    znb_t = slp.tile([HD, SW], BF16, tag="znb", name="znbt")
                    nc.vector.tensor_copy(zna[:, :], zsb[0][0:HD, :])
                    nc.vector.tensor_copy(znb_t[:, :], zsb[1][0:HD, :])
                    ltp = ps.tile([128, 8], F32, tag="sg", name="ltp")
                    for h in range(2):
                        for j in range(4):
                            nc.tensor.transpose(
                                ltp[:, h * 4 + j : h * 4 + j + 1],
                                zsb[h][HD : HD + 1, j * 128 : (j + 1) * 128],
                                ident_sb[HD : HD + 1, HD : HD + 1],
                            )
                    rinv = slp.tile([128, 8], F32, tag="rinv", name="rinv")
                    nc.vector.reciprocal(rinv[:, :], ltp[:, :])
                    for j in range(4):
                        ps_oa = ps.tile([128, D], F32, tag="sg", name="ps_oa")
                        nc.tensor.matmul(
                            ps_oa[:, :],
                            lhsT=zna[:, j * 128 : (j + 1) * 128],
                            rhs=wo_sb[0:HD, :],
                            start=True,
                            stop=True,
                            skip_group_check=True,
                        )
                        ps_ob = ps.tile([128, D], F32, tag="sg", name="ps_ob")
                        nc.tensor.matmul(
                            ps_ob[:, :],
                            lhsT=znb_t[:, j * 128 : (j + 1) * 128],
                            rhs=wob_sb[:, :],
                            start=True,
                            stop=True,
                            skip_group_check=True,
                        )
                        o_sb = outp.tile([128, D], F32, tag="ot", name="o_sb")
                        nc.vector.tensor_scalar_mul(
                            o_sb[:, :], ps_oa[:, :], rinv[:, j : j + 1]
                        )
                        nc.vector.scalar_tensor_tensor(
                            o_sb[:, :],
                            ps_ob[:, :],
                            rinv[:, 4 + j : 4 + j + 1],
                            o_sb[:, :],
                            op0=mybir.AluOpType.mult,
                            op1=mybir.AluOpType.add,
                        )
                        r0 = qs + j * 128
                        nc.sync.dma_start(out=out[r0 : r0 + 128, :], in_=o_sb[:, :])
                    continue

                znpair = slp.tile([128, SW], BF16, tag="zn", name="znpair")
                znb = slp.tile([HD, SW], BF16, tag="znb", name="znb")
                for h in range(2):
                    rd = drp.tile([1, SW], F32, tag=f"rd{h}", name="rd")
                    nc.sync.dma_start(out=rd[:, :], in_=zsb[h][HD : HD + 1, :])
                    lsp = slp.tile([128, SW // 128], F32, tag=f"lsp{h}", name="lsp")
                    nc.sync.dma_start(
                        out=lsp[:, :],
                        in_=rd[0, :].rearrange("(p f) -> p f", p=128),
                    )
                    rsp = slp.tile([128, SW // 128], F32, tag=f"rsp{h}", name="rsp")
                    nc.vector.reciprocal(rsp[:, :], lsp[:, :])
                    rd2 = drp.tile([1, SW], F32, tag=f"rd2{h}", name="rd2")
                    nc.sync.dma_start(
                        out=rd2[0, :].rearrange("(p f) -> p f", p=128),
                        in_=rsp[:, :],
                    )
                    rap = rd2[:, :]
                    bcast_src = bass.AP(
                        tensor=rap.tensor,
                        offset=rap.offset,
                        ap=[[0, HD]] + list(rap.ap[1:]),
                    )
                    bc = slp.tile([HD, SW], F32, tag=f"bc{h}", name="bc")
                    nc.sync.dma_start(out=bc[:, :], in_=bcast_src)
                    dst = znpair[0:HD, :] if h == 0 else znb[:, :]
                    nc.vector.tensor_mul(dst, zsb[h][0:HD, :], bc[:, :])
                # move head B rows into partitions 64..127
                nc.gpsimd.dma_start(out=znpair[HD:128, :], in_=znb[:, :])

                # earlier slices' O-projections: emitted after this slice's
                # attention+normalisation so their PSUM->SBUF copies never
                # delay the normalisation DVE work or stall the PE stream
                if s >= 2:
                    for p in pending:
                        emit_oproj(*p)
                    pending = []
                pending.append((znpair, qs))

            for p in pending:
                emit_oproj(*p)

    _split_waits(nc)
    return nc


_NC_CACHE = {}


def _get_nc():
    if "nc" not in _NC_CACHE:
        _NC_CACHE["nc"] = build_nc()
    return _NC_CACHE["nc"]


def make_in_maps(combined_embed, W_K, b_K, W_Q, b_Q, W_V, b_V, W_O, b_O):
    f32 = np.float32
    in_maps = []
    for c in range(8):
        b = c // 4
        g = c % 4
        sl = slice(g * 128, (g + 1) * 128)
        xt = np.ascontiguousarray(np.asarray(combined_embed[b], f32).T)
        in_maps.append(
            {
                "xt": xt.astype(_BF16),
                "wq": np.ascontiguousarray(np.asarray(W_Q, f32)[:, sl]).astype(_BF16),
                "wk": np.ascontiguousarray(np.asarray(W_K, f32)[:, sl]).astype(_BF16),
                "wv": np.ascontiguousarray(np.asarray(W_V, f32)[:, sl]).astype(_BF16),
                "wo": np.ascontiguousarray(np.asarray(W_O, f32)[sl, :]).astype(_BF16),
                "bq": np.asarray(b_Q, f32)[sl].reshape(128, 1).copy(),
                "bk": np.asarray(b_K, f32)[sl].reshape(128, 1).copy(),
                "bv": np.asarray(b_V, f32)[sl].reshape(1, 128).copy(),
            }
        )
    return in_maps


def run_cores(in_maps, **kwargs):
    nc = _get_nc()
    return run_bass_kernel_spmd(nc, in_maps, core_ids=list(range(8)), **kwargs)


def kernel(
    combined_embed, W_K, b_K, W_Q, b_Q, W_V, b_V, W_O, b_O
):  # full inputs -> full output
    in_maps = make_in_maps(
        combined_embed, W_K, b_K, W_Q, b_Q, W_V, b_V, W_O, b_O
    )
    res = run_cores(in_maps)
    out = np.zeros((B, T, D), np.float32)
    for c in range(8):
        out[c // 4] += res.results[c]["out"]
    out += np.asarray(b_O, np.float32)[None, None, :]
    return out


# revision 21
# speedup vs baseline: 1.5724x; 1.0032x over previous
"""Multi-head causal self-attention on 8 TRN2 NeuronCores.

Problem: B=2, T=4096, D=512, H=8 heads (hd=64), fp32 in/out.

Sharding: core c in 0..7 handles batch b = c//4 and head pair g = c%4
(heads 2g, 2g+1 -> D-slice [128g, 128g+128)). Each core computes
    partial_out = concat_h( softmax(causal(Q_h K_h^T / 8)) V_h ) @ W_O[slice]
for its two heads; the host sums the 4 partials per batch and adds b_O.

On-core dataflow (all matmul operands bf16, f32 PSUM accumulation):
  - X^T (host-pretransposed) streams in as 4 [128,4096] chunks.
  - Q^T,K^T [128(d-pair),4096] = W_chunk^T @ X^T, bias added during the
    PSUM->SBUF bf16 copy (per-partition scalar add on DVE).
  - V [4096,64+1] per head in natural layout (ones column appended ->
    the attention row-sum L falls out of the PV matmul for free); the
    V bias rides on the PSUM->SBUF evacuation (DVE tensor_tensor add
    against a partition-broadcast bias tile) so the PE never sees it.
  - Scores are computed transposed, S^T[k-block, q] (contraction over the
    64-dim head axis; the two heads run in disjoint PE row groups), causally
    streamed: for key block kb only q >= 128*kb is computed. exp() runs on
    ScalarE straight out of PSUM with the 1/8 scale folded in; the diagonal
    128x128 subtile is masked AFTER the exp by a DVE multiply with a 0/1
    lower-triangle tile (keeps the PE stream free of mask matmuls and of
    tiling-mode switches mid-group).
  - Z^T_aug[65, q] accumulates P^T-block x V_aug over key blocks in PSUM;
    row 64 is L. The S->exp->PV chain is software-pipelined: scores run one
    group ahead of the PV matmuls; score PSUM groups are double-buffered and
    Z^T PSUM is double-buffered across slices so the PE never waits on the
    normalisation chain.
  - Normalisation (fully off the PE): head B's Z rows are DMA-shifted from
    PSUM into partitions 64..127 of an f32 staging tile while the L rows
    bounce through DRAM into a [128,4] partition-spread layout (DVE
    iterative divide costs freedim x 8 cycles, so spread the 512 elements),
    are reciprocal'd, bounced back, and broadcast-read into a [128,512]
    tile (step-0 partition APs are only legal on DRAM sources). Two DVE
    multiplies then produce the normalised bf16 [128,512] O-proj lhsT
    directly (head A straight out of PSUM).
  - O-projection: lhsT = stacked [Z_A; Z_B] [128, t-tile], rhs = W_O pair
    [128,512]. Each slice's O-projection is emitted at the END of the next
    slice's body so its normalisation chain never stalls the PE stream and
    its PSUM->SBUF copies never delay the next slice's DVE work.
"""

import numpy as np

import concourse.bass as bass
import concourse.mybir as mybir
from concourse.tile import TileContext
from concourse.bass_utils import run_bass_kernel_spmd

try:
    import ml_dtypes

    _BF16 = ml_dtypes.bfloat16
except ImportError:  # pragma: no cover
    _BF16 = None

F32 = mybir.dt.float32
BF16 = mybir.dt.bfloat16

B, T, D, H = 2, 4096, 512, 8
HD = D // H  # 64
SW = 512  # q-slice width
NS = T // SW  # 8 q-slices
NKC = D // 128  # 4 contraction chunks for the projections
NTT = T // 128  # 32 t-tiles / key blocks
GK = 2  # key blocks grouped per exp() call (2 PSUM banks)


def _split_waits(nc, max_waits=1):
    """The staged walrus rejects >1 semaphore wait per instruction; hoist
    extras onto same-engine NoOps inserted right before the instruction."""
    counter = 0
    for f in nc.m.functions:
        for blk in f.blocks:
            insts = blk.instructions
            out, changed = [], False
            for ins in insts:
                si = getattr(ins, "sync_info", None)
                waits = list(si.on_wait) if si is not None and si.on_wait else []
                if len(waits) > max_waits:
                    changed = True
                    for w in waits[:-max_waits]:
                        counter += 1
                        nop = mybir.InstNoOp(
                            name=f"I-wsplit-{counter}",
                            engine=ins.engine,
                            ins=[],
                            outs=[],
                        )
                        nop.sync_info = mybir.SyncInfo(on_wait=[w], on_update=[])
                        out.append(nop)
                    ins.sync_info = mybir.SyncInfo(
                        on_wait=waits[-max_waits:], on_update=list(si.on_update)
                    )
                out.append(ins)
            if changed:
                blk.instructions = out
    return counter


def build_nc():
    nc = bass.Bass("TRN2")

    xt = nc.dram_tensor("xt", [D, T], BF16, kind="ExternalInput")
    wq = nc.dram_tensor("wq", [D, 128], BF16, kind="ExternalInput")
    wk = nc.dram_tensor("wk", [D, 128], BF16, kind="ExternalInput")
    wv = nc.dram_tensor("wv", [D, 128], BF16, kind="ExternalInput")
    wo = nc.dram_tensor("wo", [128, D], BF16, kind="ExternalInput")
    bq = nc.dram_tensor("bq", [128, 1], F32, kind="ExternalInput")
    bk = nc.dram_tensor("bk", [128, 1], F32, kind="ExternalInput")
    bv = nc.dram_tensor("bv", [1, 128], F32, kind="ExternalInput")
    out = nc.dram_tensor("out", [T, D], F32, kind="ExternalOutput")

    # tri01[k, q'] = 1 where q' >= k else 0  (post-exp S^T diagonal mask)
    tri_np = np.where(
        np.arange(128)[None, :] >= np.arange(128)[:, None], 1.0, 0.0
    ).astype(np.float32)
    tri_dram = nc.inline_tensor(tri_np.astype(_BF16), name="tric")
    ident_dram = nc.inline_tensor(np.eye(128, dtype=np.float32), name="identc")

    with TileContext(nc) as tc:
        with (
            tc.tile_pool(name="singles", bufs=1) as singles,
            tc.tile_pool(name="ps", bufs=3, space="PSUM") as ps,
            tc.tile_pool(name="zps", bufs=1, space="PSUM") as zps,
            tc.tile_pool(name="pt", bufs=6) as ptp,
            tc.tile_pool(name="sl", bufs=3) as slp,
            tc.tile_pool(name="outp", bufs=4) as outp,
            tc.tile_pool(name="drp", bufs=2, space="DRAM") as drp,
        ):
            # ---- static SBUF ----
            # DMA issue is serialized per queue (~0.7us each), so: the handful
            # the first projections need go on the sync queue first (QKV
            # weights as ONE consolidated DMA each + slice 0 of X^T), the X^T
            # bulk goes on the gpsimd queue, everything else on scalar/vector.
            xt_sb = [
                singles.tile([128, NS, SW], BF16, tag=f"xt{c}", name=f"xt_sb{c}")
                for c in range(NKC)
            ]
            wq_sb = singles.tile([128, NKC, 128], BF16, tag="wq")
            wk_sb = singles.tile([128, NKC, 128], BF16, tag="wk")
            wv_sb = singles.tile([128, NKC, 128], BF16, tag="wv")
            nc.sync.dma_start(
                out=wq_sb[:, :, :],
                in_=wq[:, :].rearrange("(c p) f -> p c f", p=128),
            )
            nc.sync.dma_start(
                out=wk_sb[:, :, :],
                in_=wk[:, :].rearrange("(c p) f -> p c f", p=128),
            )
            nc.sync.dma_start(
                out=wv_sb[:, :, :],
                in_=wv[:, :].rearrange("(c p) f -> p c f", p=128),
            )
            for c in range(NKC):
                nc.sync.dma_start(
                    out=xt_sb[c][:, 0, :],
                    in_=xt[c * 128 : (c + 1) * 128, 0:SW],
                )
            for si in range(1, NS):
                for c in range(NKC):
                    nc.gpsimd.dma_start(
                        out=xt_sb[c][:, si, :],
                        in_=xt[c * 128 : (c + 1) * 128, si * SW : (si + 1) * SW],
                    )

            bq_sb = singles.tile([128, 1], F32, tag="bq")
            bk_sb = singles.tile([128, 1], F32, tag="bk")
            nc.scalar.dma_start(out=bq_sb[:, :], in_=bq[:, :])
            nc.scalar.dma_start(out=bk_sb[:, :], in_=bk[:, :])

            wo_sb = singles.tile([128, D], BF16, tag="wo")
            nc.scalar.dma_start(out=wo_sb[:, :], in_=wo[:, :])
            # W_O rows 64..127 restaged at partitions 0..63 for the tail's
            # per-head 64-contraction O-projections
            wob_sb = singles.tile([HD, D], BF16, tag="wob")
            nc.scalar.dma_start(out=wob_sb[:, :], in_=wo[HD:128, :])
            # b_V broadcast across partitions: [128, 128] f32, every row = b_V
            bvb_sb = singles.tile([128, 128], F32, tag="bvb")
            bv_ap = bv[:, :]
            bvb_src = bass.AP(
                tensor=bv_ap.tensor,
                offset=bv_ap.offset,
                ap=[[0, 128]] + list(bv_ap.ap[1:]),
            )
            nc.scalar.dma_start(out=bvb_sb[:, :], in_=bvb_src)

            tri_sb = singles.tile([128, 128], BF16, tag="tri")
            nc.scalar.dma_start(out=tri_sb[:, :], in_=tri_dram[:, :])
            ident_sb = singles.tile([128, 128], F32, tag="ident")
            nc.scalar.dma_start(out=ident_sb[:, :], in_=ident_dram[:, :])

            qt_sb = [
                singles.tile([128, SW], BF16, tag=f"qt{s}", name=f"qt_sb{s}")
                for s in range(NS)
            ]
            kt_sb = [
                singles.tile([128, SW], BF16, tag=f"kt{s}", name=f"kt_sb{s}")
                for s in range(NS)
            ]
            # partition-swapped copies (head B rows 0..63, head A rows
            # 64..127): odd key blocks run head A in PE rows 64-127 and head
            # B in rows 0-63, so consecutive score matmuls always hit
            # disjoint row groups and their LDWEIGHTS pull ahead
            qw_sb = [
                singles.tile([128, SW], BF16, tag=f"qw{s}", name=f"qw_sb{s}")
                for s in range(NS)
            ]
            kw_sb = [
                singles.tile([128, SW], BF16, tag=f"kw{s}", name=f"kw_sb{s}")
                for s in range(NS)
            ]
            # V_aug per head per key block: [128(t), 65]; col 64 = ones
            va_sb = [
                singles.tile([128, HD + 1], BF16, tag=f"va{t}", name=f"va_sb{t}")
                for t in range(NTT)
            ]
            vb_sb = [
                singles.tile([128, HD + 1], BF16, tag=f"vb{t}", name=f"vb_sb{t}")
                for t in range(NTT)
            ]

            # ---- QKV projections (emitted per q-slice, interleaved with
            # attention so ScalarE starts exp-ing early) ----
            def emit_qkv(s):
                ps_q = ps.tile([128, SW], F32, tag="sg", name="ps_q")
                for c in range(NKC):
                    nc.tensor.matmul(
                        ps_q[:, :],
                        lhsT=wq_sb[:, c, :],
                        rhs=xt_sb[c][:, s, :],
                        start=(c == 0),
                        stop=(c == NKC - 1),
                        skip_group_check=True,
                    )
                nc.vector.tensor_scalar_add(qt_sb[s][:, :], ps_q[:, :], bq_sb[:, :])
                nc.gpsimd.dma_start(out=qw_sb[s][0:HD, :], in_=qt_sb[s][HD:128, :])
                nc.gpsimd.dma_start(out=qw_sb[s][HD:128, :], in_=qt_sb[s][0:HD, :])
                ps_k = ps.tile([128, SW], F32, tag="sg", name="ps_k")
                for c in range(NKC):
                    nc.tensor.matmul(
                        ps_k[:, :],
                        lhsT=wk_sb[:, c, :],
                        rhs=xt_sb[c][:, s, :],
                        start=(c == 0),
                        stop=(c == NKC - 1),
                        skip_group_check=True,
                    )
                nc.vector.tensor_scalar_add(kt_sb[s][:, :], ps_k[:, :], bk_sb[:, :])
                nc.gpsimd.dma_start(out=kw_sb[s][0:HD, :], in_=kt_sb[s][HD:128, :])
                nc.gpsimd.dma_start(out=kw_sb[s][HD:128, :], in_=kt_sb[s][0:HD, :])
                for t in range(4 * s, 4 * s + 4):
                    tloc = slice((t % 4) * 128, (t % 4 + 1) * 128)
                    ps_v = ps.tile([128, 128], F32, tag="sg", name="ps_v")
                    for c in range(NKC):
                        nc.tensor.matmul(
                            ps_v[:, :],
                            lhsT=xt_sb[c][:, s, tloc],
                            rhs=wv_sb[:, c, :],
                            start=(c == 0),
                            stop=(c == NKC - 1),
                            skip_group_check=True,
                        )
                    nc.vector.tensor_tensor(
                        va_sb[t][:, 0:HD],
                        ps_v[:, 0:HD],
                        bvb_sb[:, 0:HD],
                        op=mybir.AluOpType.add,
                    )
                    nc.vector.tensor_tensor(
                        vb_sb[t][:, 0:HD],
                        ps_v[:, HD:128],
                        bvb_sb[:, HD:128],
                        op=mybir.AluOpType.add,
                    )
                    nc.vector.memset(va_sb[t][:, HD : HD + 1], 1.0)
                    nc.vector.memset(vb_sb[t][:, HD : HD + 1], 1.0)

            # ---- attention ----
            vmat = (va_sb, vb_sb)
            hrows = (slice(0, HD), slice(HD, 128))

            def emit_oproj(znpair_t, qs_t):
                for j in range(4):
                    ps_o = ps.tile([128, D], F32, tag="sg", name="ps_o")
                    nc.tensor.matmul(
                        ps_o[:, :],
                        lhsT=znpair_t[:, j * 128 : (j + 1) * 128],
                        rhs=wo_sb[:, :],
                        start=True,
                        stop=True,
                        skip_group_check=True,
                    )
                    o_sb = outp.tile([128, D], F32, tag="ot", name="o_sb")
                    nc.vector.tensor_copy(o_sb[:, :], ps_o[:, :])
                    r0 = qs_t + j * 128
                    nc.sync.dma_start(out=out[r0 : r0 + 128, :], in_=o_sb[:, :])

            pending = []
            for s in range(NS):
                emit_qkv(s)
                qs = s * SW
                nkb = 4 * (s + 1)
                zaug = [
                    zps.tile([HD + 1, SW], F32, tag="za", name="zauga"),
                    zps.tile([HD + 1, SW], F32, tag="zb", name="zaugb"),
                ]

                def emit_av(av):
                    pt_t, kb_t, n_t, qlo_t = av
                    for h in range(2):
                        nc.tensor.matmul(
                            zaug[h][0 : HD + 1, qlo_t - qs : SW],
                            lhsT=vmat[h][kb_t][:, :],
                            rhs=pt_t[:, h, 0:n_t],
                            start=(kb_t == 0),
                            stop=(kb_t == nkb - 1),
                            skip_group_check=True,
                        )

                # head-major score groups: one [128, 2, 512] PSUM tile per
                # key block (head A in bank 0, head B in bank 1) -> three
                # kb-groups in flight through the S -> exp -> PV pipeline
                av_queue = []
                for kb in range(nkb):
                    qlo = max(qs, kb * 128)
                    n = qs + SW - qlo
                    diag = kb * 128 >= qs
                    sg = ps.tile([128, 2, SW], F32, tag="sg", name="sg")
                    pt = ptp.tile([128, 2, SW], BF16, tag="pt", name="pt")
                    # scores: both heads in disjoint PE row groups, with
                    # the head<->row-group assignment alternating per key
                    # block so back-to-back matmuls never share row groups
                    kcol = slice((kb % 4) * 128, (kb % 4 + 1) * 128)
                    qcol = slice(qlo - qs, qlo - qs + n)
                    if kb % 2 == 0:
                        plan = [(0, kt_sb[kb // 4], qt_sb[s], hrows[0], 0),
                                (1, kt_sb[kb // 4], qt_sb[s], hrows[1], HD)]
                    else:
                        plan = [(1, kw_sb[kb // 4], qw_sb[s], hrows[0], 0),
                                (0, kw_sb[kb // 4], qw_sb[s], hrows[1], HD)]
                    for h, ksrc, qsrc, rows, tp in plan:
                        nc.tensor.matmul(
                            sg[:, h, 0:n],
                            lhsT=ksrc[rows, kcol],
                            rhs=qsrc[rows, qcol],
                            start=True,
                            stop=True,
                            skip_group_check=True,
                            tile_position=(tp, 0),
                        )
                    nc.scalar.activation(
                        out=pt[:, :, 0:n],
                        in_=sg[:, :, 0:n],
                        func=mybir.ActivationFunctionType.Exp,
                        scale=0.125,
                    )
                    if diag:
                        # post-exp causal mask on the diagonal 128x128 subtile
                        for h in range(2):
                            nc.vector.tensor_mul(
                                pt[:, h, 0:128],
                                pt[:, h, 0:128],
                                tri_sb[:, :],
                            )
                    av_queue.append((pt, kb, n, qlo))
                    if len(av_queue) > 2:
                        emit_av(av_queue.pop(0))
                while av_queue:
                    emit_av(av_queue.pop(0))

                # ---- normalisation (no PE work; zaug evacuated by ScalarE
                # right after the last PV so the single-buffered Z PSUM bank
                # frees before the next slice's PV needs it) ----
                zsb = [None, None]
                for h in range(2):
                    zsb[h] = slp.tile([HD + 1, SW], F32, tag=f"zsb{h}", name="zsb")
                    nc.scalar.activation(
                        out=zsb[h][:, :],
                        in_=zaug[h][:, :],
                        func=mybir.ActivationFunctionType.Copy,
                    )

                if s == NS - 1:
                    # ---- low-latency tail: skip the DRAM 1/L bounce.
                    # L rows -> per-partition layout via PE transposes (PE is
                    # idle here), one DVE reciprocal, then per-head 64-contraction
                    # O-projections whose 1/L scale rides on the PSUM->SBUF
                    # evacuation (tensor_scalar ops, per-partition scalars).
                    if pending:  # give the PE work while the chain drains
                        for p in pending:
                            emit_oproj(*p)
                        pending = []
                    zna = slp.tile([HD, SW], BF16, tag="zn", name="zna")
                    znb_t = slp.tile([HD, SW], BF16, tag="znb", name="znbt")
                    nc.vector.tensor_copy(zna[:, :], zsb[0][0:HD, :])
                    nc.vector.tensor_copy(znb_t[:, :], zsb[1][0:HD, :])
                    ltp = ps.tile([128, 8], F32, tag="sg", name="ltp")
                    for h in range(2):
                        for j in range(4):
                            nc.tensor.transpose(
                                ltp[:, h * 4 + j : h * 4 + j + 1],
                                zsb[h][HD : HD + 1, j * 128 : (j + 1) * 128],
                                ident_sb[HD : HD + 1, HD : HD + 1],
                            )
                    rinv = slp.tile([128, 8], F32, tag="rinv", name="rinv")
                    nc.vector.reciprocal(rinv[:, :], ltp[:, :])
                    for j in range(4):
                        ps_oa = ps.tile([128, D], F32, tag="sg", name="ps_oa")
                        nc.tensor.matmul(
                            ps_oa[:, :],
                            lhsT=zna[:, j * 128 : (j + 1) * 128],
                            rhs=wo_sb[0:HD, :],
                            start=True,
                            stop=True,
                            skip_group_check=True,
                        )
                        ps_ob = ps.tile([128, D], F32, tag="sg", name="ps_ob")
                        nc.tensor.matmul(
                            ps_ob[:, :],
                            lhsT=znb_t[:, j * 128 : (j + 1) * 128],
                            rhs=wob_sb[:, :],
                            start=True,
                            stop=True,
                            skip_group_check=True,
                        )
                        o_sb = outp.tile([128, D], F32, tag="ot", name="o_sb")
                        nc.vector.tensor_scalar_mul(
                            o_sb[:, :], ps_oa[:, :], rinv[:, j : j + 1]
                        )
                        nc.vector.scalar_tensor_tensor(
                            o_sb[:, :],
                            ps_ob[:, :],
                            rinv[:, 4 + j : 4 + j + 1],
                            o_sb[:, :],
                            op0=mybir.AluOpType.mult,
                            op1=mybir.AluOpType.add,
                        )
                        r0 = qs + j * 128
                        nc.sync.dma_start(out=out[r0 : r0 + 128, :], in_=o_sb[:, :])
                    continue

                znpair = slp.tile([128, SW], BF16, tag="zn", name="znpair")
                znb = slp.tile([HD, SW], BF16, tag="znb", name="znb")
                for h in range(2):
                    rd = drp.tile([1, SW], F32, tag=f"rd{h}", name="rd")
                    nc.sync.dma_start(out=rd[:, :], in_=zsb[h][HD : HD + 1, :])
                    lsp = slp.tile([128, SW // 128], F32, tag=f"lsp{h}", name="lsp")
                    nc.sync.dma_start(
                        out=lsp[:, :],
                        in_=rd[0, :].rearrange("(p f) -> p f", p=128),
                    )
                    rsp = slp.tile([128, SW // 128], F32, tag=f"rsp{h}", name="rsp")
                    nc.vector.reciprocal(rsp[:, :], lsp[:, :])
                    rd2 = drp.tile([1, SW], F32, tag=f"rd2{h}", name="rd2")
                    nc.sync.dma_start(
                        out=rd2[0, :].rearrange("(p f) -> p f", p=128),
                        in_=rsp[:, :],
                    )
                    rap = rd2[:, :]
                    bcast_src = bass.AP(
                        tensor=rap.tensor,
                        offset=rap.offset,
                        ap=[[0, HD]] + list(rap.ap[1:]),
                    )
                    bc = slp.tile([HD, SW], F32, tag=f"bc{h}", name="bc")
                    nc.sync.dma_start(out=bc[:, :], in_=bcast_src)
                    dst = znpair[0:HD, :] if h == 0 else znb[:, :]
                    nc.vector.tensor_mul(dst, zsb[h][0:HD, :], bc[:, :])
                # move head B rows into partitions 64..127
                nc.gpsimd.dma_start(out=znpair[HD:128, :], in_=znb[:, :])

                # earlier slices' O-projections: emitted after this slice's
                # attention+normalisation so their PSUM->SBUF copies never
                # delay the normalisation DVE work or stall the PE stream
                if s >= 2:
                    for p in pending:
                        emit_oproj(*p)
                    pending = []
                pending.append((znpair, qs))

            for p in pending:
                emit_oproj(*p)

    _split_waits(nc)
    return nc


_NC_CACHE = {}


def _get_nc():
    if "nc" not in _NC_CACHE:
        _NC_CACHE["nc"] = build_nc()
    return _NC_CACHE["nc"]


def make_in_maps(combined_embed, W_K, b_K, W_Q, b_Q, W_V, b_V, W_O, b_O):
    f32 = np.float32
    in_maps = []
    for c in range(8):
        b = c // 4
        g = c % 4
        sl = slice(g * 128, (g + 1) * 128)
        xt = np.ascontiguousarray(np.asarray(combined_embed[b], f32).T)
        in_maps.append(
            {
                "xt": xt.astype(_BF16),
                "wq": np.ascontiguousarray(np.asarray(W_Q, f32)[:, sl]).astype(_BF16),
                "wk": np.ascontiguousarray(np.asarray(W_K, f32)[:, sl]).astype(_BF16),
                "wv": np.ascontiguousarray(np.asarray(W_V, f32)[:, sl]).astype(_BF16),
                "wo": np.ascontiguousarray(np.asarray(W_O, f32)[sl, :]).astype(_BF16),
                "bq": np.asarray(b_Q, f32)[sl].reshape(128, 1).copy(),
                "bk": np.asarray(b_K, f32)[sl].reshape(128, 1).copy(),
                "bv": np.asarray(b_V, f32)[sl].reshape(1, 128).copy(),
            }
        )
    return in_maps


def run_cores(in_maps, **kwargs):
    nc = _get_nc()
    return run_bass_kernel_spmd(nc, in_maps, core_ids=list(range(8)), **kwargs)


def kernel(
    combined_embed, W_K, b_K, W_Q, b_Q, W_V, b_V, W_O, b_O
):  # full inputs -> full output
    in_maps = make_in_maps(
        combined_embed, W_K, b_K, W_Q, b_Q, W_V, b_V, W_O, b_O
    )
    res = run_cores(in_maps)
    out = np.zeros((B, T, D), np.float32)
    for c in range(8):
        out[c // 4] += res.results[c]["out"]
    out += np.asarray(b_O, np.float32)[None, None, :]
    return out


# revision 22
# speedup vs baseline: 1.6257x; 1.0339x over previous
"""Multi-head causal self-attention on 8 TRN2 NeuronCores.

Problem: B=2, T=4096, D=512, H=8 heads (hd=64), fp32 in/out.

Sharding: core c in 0..7 handles batch b = c//4 and head pair g = c%4
(heads 2g, 2g+1 -> D-slice [128g, 128g+128)). Each core computes
    partial_out = concat_h( softmax(causal(Q_h K_h^T / 8)) V_h ) @ W_O[slice]
for its two heads; the host sums the 4 partials per batch and adds b_O.

On-core dataflow (all matmul operands bf16, f32 PSUM accumulation):
  - X^T (host-pretransposed) streams in as 4 [128,4096] chunks.
  - Q^T,K^T [128(d-pair),4096] = W_chunk^T @ X^T, bias added during the
    PSUM->SBUF bf16 copy (per-partition scalar add on DVE).
  - V [4096,64+1] per head in natural layout (ones column appended ->
    the attention row-sum L falls out of the PV matmul for free); the
    V bias rides on the PSUM->SBUF evacuation (DVE tensor_tensor add
    against a partition-broadcast bias tile) so the PE never sees it.
  - Scores are computed transposed, S^T[k-block, q] (contraction over the
    64-dim head axis; the two heads run in disjoint PE row groups), causally
    streamed: for key block kb only q >= 128*kb is computed. exp() runs on
    ScalarE straight out of PSUM with the 1/8 scale folded in; the diagonal
    128x128 subtile is masked AFTER the exp by a DVE multiply with a 0/1
    lower-triangle tile (keeps the PE stream free of mask matmuls and of
    tiling-mode switches mid-group).
  - Z^T_aug[65, q] accumulates P^T-block x V_aug over key blocks in PSUM;
    row 64 is L. The S->exp->PV chain is software-pipelined: scores run one
    group ahead of the PV matmuls; score PSUM groups are double-buffered and
    Z^T PSUM is double-buffered across slices so the PE never waits on the
    normalisation chain.
  - Normalisation (fully off the PE): head B's Z rows are DMA-shifted from
    PSUM into partitions 64..127 of an f32 staging tile while the L rows
    bounce through DRAM into a [128,4] partition-spread layout (DVE
    iterative divide costs freedim x 8 cycles, so spread the 512 elements),
    are reciprocal'd, bounced back, and broadcast-read into a [128,512]
    tile (step-0 partition APs are only legal on DRAM sources). Two DVE
    multiplies then produce the normalised bf16 [128,512] O-proj lhsT
    directly (head A straight out of PSUM).
  - O-projection: lhsT = stacked [Z_A; Z_B] [128, t-tile], rhs = W_O pair
    [128,512]. Each slice's O-projection is emitted at the END of the next
    slice's body so its normalisation chain never stalls the PE stream and
    its PSUM->SBUF copies never delay the next slice's DVE work.
"""

import numpy as np

import concourse.bass as bass
import concourse.mybir as mybir
from concourse.tile import TileContext
from concourse.bass_utils import run_bass_kernel_spmd

try:
    import ml_dtypes

    _BF16 = ml_dtypes.bfloat16
except ImportError:  # pragma: no cover
    _BF16 = None

F32 = mybir.dt.float32
BF16 = mybir.dt.bfloat16

B, T, D, H = 2, 4096, 512, 8
HD = D // H  # 64
SW = 512  # q-slice width
NS = T // SW  # 8 q-slices
NKC = D // 128  # 4 contraction chunks for the projections
NTT = T // 128  # 32 t-tiles / key blocks
GK = 2  # key blocks grouped per exp() call (2 PSUM banks)


def _split_waits(nc, max_waits=1):
    """The staged walrus rejects >1 semaphore wait per instruction; hoist
    extras onto same-engine NoOps inserted right before the instruction."""
    counter = 0
    for f in nc.m.functions:
        for blk in f.blocks:
            insts = blk.instructions
            out, changed = [], False
            for ins in insts:
                si = getattr(ins, "sync_info", None)
                waits = list(si.on_wait) if si is not None and si.on_wait else []
                if len(waits) > max_waits:
                    changed = True
                    for w in waits[:-max_waits]:
                        counter += 1
                        nop = mybir.InstNoOp(
                            name=f"I-wsplit-{counter}",
                            engine=ins.engine,
                            ins=[],
                            outs=[],
                        )
                        nop.sync_info = mybir.SyncInfo(on_wait=[w], on_update=[])
                        out.append(nop)
                    ins.sync_info = mybir.SyncInfo(
                        on_wait=waits[-max_waits:], on_update=list(si.on_update)
                    )
                out.append(ins)
            if changed:
                blk.instructions = out
    return counter


def build_nc():
    nc = bass.Bass("TRN2")

    xt = nc.dram_tensor("xt", [D, T], BF16, kind="ExternalInput")
    wq = nc.dram_tensor("wq", [D, 128], BF16, kind="ExternalInput")
    wk = nc.dram_tensor("wk", [D, 128], BF16, kind="ExternalInput")
    wv = nc.dram_tensor("wv", [D, 128], BF16, kind="ExternalInput")
    wo = nc.dram_tensor("wo", [128, D], BF16, kind="ExternalInput")
    bq = nc.dram_tensor("bq", [128, 1], F32, kind="ExternalInput")
    bk = nc.dram_tensor("bk", [128, 1], F32, kind="ExternalInput")
    bv = nc.dram_tensor("bv", [1, 128], F32, kind="ExternalInput")
    out = nc.dram_tensor("out", [T, D], F32, kind="ExternalOutput")

    # tri01[k, q'] = 1 where q' >= k else 0  (post-exp S^T diagonal mask)
    tri_np = np.where(
        np.arange(128)[None, :] >= np.arange(128)[:, None], 1.0, 0.0
    ).astype(np.float32)
    tri_dram = nc.inline_tensor(tri_np.astype(_BF16), name="tric")
    ident_dram = nc.inline_tensor(np.eye(128, dtype=np.float32), name="identc")

    with TileContext(nc) as tc:
        with (
            tc.tile_pool(name="singles", bufs=1) as singles,
            tc.tile_pool(name="ps", bufs=3, space="PSUM") as ps,
            tc.tile_pool(name="zps", bufs=1, space="PSUM") as zps,
            tc.tile_pool(name="pt", bufs=6) as ptp,
            tc.tile_pool(name="sl", bufs=3) as slp,
            tc.tile_pool(name="outp", bufs=4) as outp,
            tc.tile_pool(name="drp", bufs=2, space="DRAM") as drp,
        ):
            # ---- static SBUF ----
            # DMA issue is serialized per queue (~0.7us each), so: the handful
            # the first projections need go on the sync queue first (QKV
            # weights as ONE consolidated DMA each + slice 0 of X^T), the X^T
            # bulk goes on the gpsimd queue, everything else on scalar/vector.
            xt_sb = [
                singles.tile([128, NS, SW], BF16, tag=f"xt{c}", name=f"xt_sb{c}")
                for c in range(NKC)
            ]
            wq_sb = singles.tile([128, NKC, 128], BF16, tag="wq")
            wk_sb = singles.tile([128, NKC, 128], BF16, tag="wk")
            wv_sb = singles.tile([128, NKC, 128], BF16, tag="wv")
            nc.sync.dma_start(
                out=wq_sb[:, :, :],
                in_=wq[:, :].rearrange("(c p) f -> p c f", p=128),
            )
            nc.sync.dma_start(
                out=wk_sb[:, :, :],
                in_=wk[:, :].rearrange("(c p) f -> p c f", p=128),
            )
            nc.sync.dma_start(
                out=wv_sb[:, :, :],
                in_=wv[:, :].rearrange("(c p) f -> p c f", p=128),
            )
            for c in range(NKC):
                nc.sync.dma_start(
                    out=xt_sb[c][:, 0, :],
                    in_=xt[c * 128 : (c + 1) * 128, 0:SW],
                )
            for si in range(1, NS):
                for c in range(NKC):
                    nc.gpsimd.dma_start(
                        out=xt_sb[c][:, si, :],
                        in_=xt[c * 128 : (c + 1) * 128, si * SW : (si + 1) * SW],
                    )

            bq_sb = singles.tile([128, 1], F32, tag="bq")
            bk_sb = singles.tile([128, 1], F32, tag="bk")
            nc.scalar.dma_start(out=bq_sb[:, :], in_=bq[:, :])
            nc.scalar.dma_start(out=bk_sb[:, :], in_=bk[:, :])

            wo_sb = singles.tile([128, D], BF16, tag="wo")
            nc.scalar.dma_start(out=wo_sb[:, :], in_=wo[:, :])
            # W_O rows 64..127 restaged at partitions 0..63 for the tail's
            # per-head 64-contraction O-projections
            wob_sb = singles.tile([HD, D], BF16, tag="wob")
            nc.scalar.dma_start(out=wob_sb[:, :], in_=wo[HD:128, :])
            # b_V broadcast across partitions: [128, 128] f32, every row = b_V
            bvb_sb = singles.tile([128, 128], F32, tag="bvb")
            bv_ap = bv[:, :]
            bvb_src = bass.AP(
                tensor=bv_ap.tensor,
                offset=bv_ap.offset,
                ap=[[0, 128]] + list(bv_ap.ap[1:]),
            )
            nc.scalar.dma_start(out=bvb_sb[:, :], in_=bvb_src)

            tri_sb = singles.tile([128, 128], BF16, tag="tri")
            nc.scalar.dma_start(out=tri_sb[:, :], in_=tri_dram[:, :])
            ident_sb = singles.tile([128, 128], F32, tag="ident")
            nc.scalar.dma_start(out=ident_sb[:, :], in_=ident_dram[:, :])

            qt_sb = [
                singles.tile([128, SW], BF16, tag=f"qt{s}", name=f"qt_sb{s}")
                for s in range(NS)
            ]
            kt_sb = [
                singles.tile([128, SW], BF16, tag=f"kt{s}", name=f"kt_sb{s}")
                for s in range(NS)
            ]

            # V_aug per head per key block: [128(t), 65]; col 64 = ones
            va_sb = [
                singles.tile([128, HD + 1], BF16, tag=f"va{t}", name=f"va_sb{t}")
                for t in range(NTT)
            ]
            vb_sb = [
                singles.tile([128, HD + 1], BF16, tag=f"vb{t}", name=f"vb_sb{t}")
                for t in range(NTT)
            ]

            # ---- QKV projections (emitted per q-slice, interleaved with
            # attention so ScalarE starts exp-ing early) ----
            def emit_qkv(s):
                ps_q = ps.tile([128, SW], F32, tag="sg", name="ps_q")
                for c in range(NKC):
                    nc.tensor.matmul(
                        ps_q[:, :],
                        lhsT=wq_sb[:, c, :],
                        rhs=xt_sb[c][:, s, :],
                        start=(c == 0),
                        stop=(c == NKC - 1),
                        skip_group_check=True,
                    )
                nc.vector.tensor_scalar_add(qt_sb[s][:, :], ps_q[:, :], bq_sb[:, :])
                ps_k = ps.tile([128, SW], F32, tag="sg", name="ps_k")
                for c in range(NKC):
                    nc.tensor.matmul(
                        ps_k[:, :],
                        lhsT=wk_sb[:, c, :],
                        rhs=xt_sb[c][:, s, :],
                        start=(c == 0),
                        stop=(c == NKC - 1),
                        skip_group_check=True,
                    )
                nc.vector.tensor_scalar_add(kt_sb[s][:, :], ps_k[:, :], bk_sb[:, :])
                for t in range(4 * s, 4 * s + 4):
                    tloc = slice((t % 4) * 128, (t % 4 + 1) * 128)
                    ps_v = ps.tile([128, 128], F32, tag="sg", name="ps_v")
                    for c in range(NKC):
                        nc.tensor.matmul(
                            ps_v[:, :],
                            lhsT=xt_sb[c][:, s, tloc],
                            rhs=wv_sb[:, c, :],
                            start=(c == 0),
                            stop=(c == NKC - 1),
                            skip_group_check=True,
                        )
                    nc.vector.tensor_tensor(
                        va_sb[t][:, 0:HD],
                        ps_v[:, 0:HD],
                        bvb_sb[:, 0:HD],
                        op=mybir.AluOpType.add,
                    )
                    nc.vector.tensor_tensor(
                        vb_sb[t][:, 0:HD],
                        ps_v[:, HD:128],
                        bvb_sb[:, HD:128],
                        op=mybir.AluOpType.add,
                    )
                    nc.vector.memset(va_sb[t][:, HD : HD + 1], 1.0)
                    nc.vector.memset(vb_sb[t][:, HD : HD + 1], 1.0)

            # ---- attention ----
            vmat = (va_sb, vb_sb)
            hrows = (slice(0, HD), slice(HD, 128))

            def emit_oproj(znpair_t, qs_t):
                for j in range(4):
                    ps_o = ps.tile([128, D], F32, tag="sg", name="ps_o")
                    nc.tensor.matmul(
                        ps_o[:, :],
                        lhsT=znpair_t[:, j * 128 : (j + 1) * 128],
                        rhs=wo_sb[:, :],
                        start=True,
                        stop=True,
                        skip_group_check=True,
                    )
                    o_sb = outp.tile([128, D], F32, tag="ot", name="o_sb")
                    nc.vector.tensor_copy(o_sb[:, :], ps_o[:, :])
                    r0 = qs_t + j * 128
                    nc.sync.dma_start(out=out[r0 : r0 + 128, :], in_=o_sb[:, :])

            pending = []
            for s in range(NS):
                emit_qkv(s)
                qs = s * SW
                nkb = 4 * (s + 1)
                zaug = [
                    zps.tile([HD + 1, SW], F32, tag="za", name="zauga"),
                    zps.tile([HD + 1, SW], F32, tag="zb", name="zaugb"),
                ]

                def emit_av(av):
                    pt_t, kb_t, n_t, qlo_t = av
                    for h in range(2):
                        nc.tensor.matmul(
                            zaug[h][0 : HD + 1, qlo_t - qs : SW],
                            lhsT=vmat[h][kb_t][:, :],
                            rhs=pt_t[:, h, 0:n_t],
                            start=(kb_t == 0),
                            stop=(kb_t == nkb - 1),
                            skip_group_check=True,
                        )

                # head-major score groups: one [128, 2, 512] PSUM tile per
                # key block (head A in bank 0, head B in bank 1) -> three
                # kb-groups in flight through the S -> exp -> PV pipeline
                av_queue = []
                for kb in range(nkb):
                    qlo = max(qs, kb * 128)
                    n = qs + SW - qlo
                    diag = kb * 128 >= qs
                    sg = ps.tile([128, 2, SW], F32, tag="sg", name="sg")
                    pt = ptp.tile([128, 2, SW], BF16, tag="pt", name="pt")
                    # scores (both heads -> disjoint PE row groups)
                    for h in range(2):
                        nc.tensor.matmul(
                            sg[:, h, 0:n],
                            lhsT=kt_sb[kb // 4][hrows[h], (kb % 4) * 128 : (kb % 4 + 1) * 128],
                            rhs=qt_sb[s][hrows[h], qlo - qs : qlo - qs + n],
                            start=True,
                            stop=True,
                            skip_group_check=True,
                            tile_position=(h * HD, 0),
                        )
                    nc.scalar.activation(
                        out=pt[:, :, 0:n],
                        in_=sg[:, :, 0:n],
                        func=mybir.ActivationFunctionType.Exp,
                        scale=0.125,
                    )
                    if diag:
                        # post-exp causal mask on the diagonal 128x128 subtile
                        for h in range(2):
                            nc.vector.tensor_mul(
                                pt[:, h, 0:128],
                                pt[:, h, 0:128],
                                tri_sb[:, :],
                            )
                    av_queue.append((pt, kb, n, qlo))
                    if len(av_queue) > 2:
                        emit_av(av_queue.pop(0))
                while av_queue:
                    emit_av(av_queue.pop(0))

                # ---- normalisation (no PE work; zaug evacuated by ScalarE
                # right after the last PV so the single-buffered Z PSUM bank
                # frees before the next slice's PV needs it) ----
                zsb = [None, None]
                for h in range(2):
                    zsb[h] = slp.tile([HD + 1, SW], F32, tag=f"zsb{h}", name="zsb")
                    nc.vector.tensor_copy(zsb[h][:, :], zaug[h][:, :])

                if s == NS - 1:
                    # ---- low-latency tail: skip the DRAM 1/L bounce.
                    # L rows -> per-partition layout via PE transposes (PE is
                    # idle here), one DVE reciprocal, then per-head 64-contraction
                    # O-projections whose 1/L scale rides on the PSUM->SBUF
                    # evacuation (tensor_scalar ops, per-partition scalars).
                    if pending:  # give the PE work while the chain drains
                        for p in pending:
                            emit_oproj(*p)
                        pending = []
                    zna = slp.tile([HD, SW], BF16, tag="zn", name="zna")
                    znb_t = slp.tile([HD, SW], BF16, tag="znb", name="znbt")
                    nc.vector.tensor_copy(zna[:, :], zsb[0][0:HD, :])
                    nc.vector.tensor_copy(znb_t[:, :], zsb[1][0:HD, :])
                    ltp = ps.tile([128, 8], F32, tag="sg", name="ltp")
                    for h in range(2):
                        for j in range(4):
                            nc.tensor.transpose(
                                ltp[:, h * 4 + j : h * 4 + j + 1],
                                zsb[h][HD : HD + 1, j * 128 : (j + 1) * 128],
                                ident_sb[HD : HD + 1, HD : HD + 1],
                            )
                    rinv = slp.tile([128, 8], F32, tag="rinv", name="rinv")
                    nc.vector.reciprocal(rinv[:, :], ltp[:, :])
                    for j in range(4):
                        ps_oa = ps.tile([128, D], F32, tag="sg", name="ps_oa")
                        nc.tensor.matmul(
                            ps_oa[:, :],
                            lhsT=zna[:, j * 128 : (j + 1) * 128],
                            rhs=wo_sb[0:HD, :],
                            start=True,
                            stop=True,
                            skip_group_check=True,
                        )
                        ps_ob = ps.tile([128, D], F32, tag="sg", name="ps_ob")
                        nc.tensor.matmul(
                            ps_ob[:, :],
                            lhsT=znb_t[:, j * 128 : (j + 1) * 128],
                            rhs=wob_sb[:, :],
                            start=True,
                            stop=True,
                            skip_group_check=True,
                        )
                        o_sb = outp.tile([128, D], F32, tag="ot", name="o_sb")
                        nc.vector.tensor_scalar_mul(
                            o_sb[:, :], ps_oa[:, :], rinv[:, j : j + 1]
                        )
                        nc.vector.scalar_tensor_tensor(
                            o_sb[:, :],
                            ps_ob[:, :],
                            rinv[:, 4 + j : 4 + j + 1],
                            o_sb[:, :],
                            op0=mybir.AluOpType.mult,
                            op1=mybir.AluOpType.add,
                        )
                        r0 = qs + j * 128
                        nc.sync.dma_start(out=out[r0 : r0 + 128, :], in_=o_sb[:, :])
                    continue

                znpair = slp.tile([128, SW], BF16, tag="zn", name="znpair")
                znb = slp.tile([HD, SW], BF16, tag="znb", name="znb")
                for h in range(2):
                    rd = drp.tile([1, SW], F32, tag=f"rd{h}", name="rd")
                    nc.sync.dma_start(out=rd[:, :], in_=zsb[h][HD : HD + 1, :])
                    lsp = slp.tile([128, SW // 128], F32, tag=f"lsp{h}", name="lsp")
                    nc.sync.dma_start(
                        out=lsp[:, :],
                        in_=rd[0, :].rearrange("(p f) -> p f", p=128),
                    )
                    rsp = slp.tile([128, SW // 128], F32, tag=f"rsp{h}", name="rsp")
                    nc.vector.reciprocal(rsp[:, :], lsp[:, :])
                    rd2 = drp.tile([1, SW], F32, tag=f"rd2{h}", name="rd2")
                    nc.sync.dma_start(
                        out=rd2[0, :].rearrange("(p f) -> p f", p=128),
                        in_=rsp[:, :],
                    )
                    rap = rd2[:, :]
                    bcast_src = bass.AP(
                        tensor=rap.tensor,
                        offset=rap.offset,
                        ap=[[0, HD]] + list(rap.ap[1:]),
                    )
                    bc = slp.tile([HD, SW], F32, tag=f"bc{h}", name="bc")
                    nc.sync.dma_start(out=bc[:, :], in_=bcast_src)
                    dst = znpair[0:HD, :] if h == 0 else znb[:, :]
                    nc.vector.tensor_mul(dst, zsb[h][0:HD, :], bc[:, :])
                # move head B rows into partitions 64..127
                nc.gpsimd.dma_start(out=znpair[HD:128, :], in_=znb[:, :])

                # earlier slices' O-projections: emitted after this slice's
                # attention+normalisation so their PSUM->SBUF copies never
                # delay the normalisation DVE work or stall the PE stream
                if s >= 2:
                    for p in pending:
                        emit_oproj(*p)
                    pending = []
                pending.append((znpair, qs))

            for p in pending:
                emit_oproj(*p)

    _split_waits(nc)
    return nc


_NC_CACHE = {}


def _get_nc():
    if "nc" not in _NC_CACHE:
        _NC_CACHE["nc"] = build_nc()
    return _NC_CACHE["nc"]


def make_in_maps(combined_embed, W_K, b_K, W_Q, b_Q, W_V, b_V, W_O, b_O):
    f32 = np.float32
    in_maps = []
    for c in range(8):
        b = c // 4
        g = c % 4
        sl = slice(g * 128, (g + 1) * 128)
        xt = np.ascontiguousarray(np.asarray(combined_embed[b], f32).T)
        in_maps.append(
            {
                "xt": xt.astype(_BF16),
                "wq": np.ascontiguousarray(np.asarray(W_Q, f32)[:, sl]).astype(_BF16),
                "wk": np.ascontiguousarray(np.asarray(W_K, f32)[:, sl]).astype(_BF16),
                "wv": np.ascontiguousarray(np.asarray(W_V, f32)[:, sl]).astype(_BF16),
                "wo": np.ascontiguousarray(np.asarray(W_O, f32)[sl, :]).astype(_BF16),
                "bq": np.asarray(b_Q, f32)[sl].reshape(128, 1).copy(),
                "bk": np.asarray(b_K, f32)[sl].reshape(128, 1).copy(),
                "bv": np.asarray(b_V, f32)[sl].reshape(1, 128).copy(),
            }
        )
    return in_maps


def run_cores(in_maps, **kwargs):
    nc = _get_nc()
    return run_bass_kernel_spmd(nc, in_maps, core_ids=list(range(8)), **kwargs)


def kernel(
    combined_embed, W_K, b_K, W_Q, b_Q, W_V, b_V, W_O, b_O
):  # full inputs -> full output
    in_maps = make_in_maps(
        combined_embed, W_K, b_K, W_Q, b_Q, W_V, b_V, W_O, b_O
    )
    res = run_cores(in_maps)
    out = np.zeros((B, T, D), np.float32)
    for c in range(8):
        out[c // 4] += res.results[c]["out"]
    out += np.asarray(b_O, np.float32)[None, None, :]
    return out
